# revision 13
# baseline (speedup 1.0000x reference)
"""Trainium2 Bass kernel for NeighborhoodNormalization.

Math: the reference builds a per-point homogeneous transform
T = [[ux,-uy,0,px],[uy,ux,0,py],[0,0,1,pz],[0,0,0,1]] (u = p/||p||),
inverts it, and applies it to 64 neighbors per point.  Closed form with
r2 = px^2+py^2, n = ||p||, a = n/r2, cx = px*a, cy = py*a, s = q - p:

    out.x =  cx*sx + cy*sy
    out.y = -cy*sx + cx*sy
    out.z =  sz

Pure data parallel over the N=8192 point axis across 8 cores.

Per-core layout: 16384 points = 128 partitions x 128 columns, partition
p = b*8 + s holds points with local n = s*128 + t.  Neighbor rows stay
contiguous in HBM per point (64*3 floats), so DMAs move [128 x W*768B]
blocks.

Steady state is DVE-bound: 4 tensor_tensor passes per group (sub, mul,
swap-mul, add) in bf16; sub/mul/add hit the 2x packed rate, the xy-swap
pass runs at ~0.8 cyc/elem (2-elem reversed runs can't pack).  Input
arrives via SWDGE fp32->bf16 cast DMA (~358 GB/s HBM cap); steady input
demand is ~320 GB/s so DMA trails just under the ceiling.

Head optimization (vs the 18us steady-start of the earlier version):
 - points load fp32 on the sync HWDGE ring, in parallel with the first
   SWDGE neighbor loads (also: fp32 coefficient math, better accuracy);
 - a dummy sqrt is the first ACT op so the Sqrt activation table loads
   during boot instead of on the critical path;
 - the coefficient chain is compressed (one full-vector square) and all
   b1/b2 coefficient-tile builds run on ACT, split into head (cols
   0-32) / tail pieces so group 0's mul only waits for the head piece;
 - b1's z=1 lanes and the n-pad zero slots are DVE memsets in the
   pre-data boot window (DVE is idle until points land);
 - groups [4,4,8,16*6,12,2,2]: small leading groups start compute as
   soon as ~0.4MB of input has landed; tiny trailing groups shorten the
   final output-DMA drain.
"""

import sys

if "/opt/trn_rl_repo" not in sys.path:
    sys.path.insert(0, "/opt/trn_rl_repo")

import numpy as np

import concourse.bass as bass
import concourse.bacc as bacc
import concourse.mybir as mybir
from concourse.tile import TileContext
from concourse.bass_utils import run_bass_kernel_spmd

B = 16
N = 8192
K = 64
NCORES = 8
NLOC = N // NCORES  # 1024 points per core
P = 128             # SBUF partitions
S = NLOC // P       # 8 partition sub-blocks per batch entry
T = (B * NLOC) // P  # 128 point-columns per partition
GMAX = 16
GROUPS = [4, 4, 8] + [16] * 6 + [12, 2, 2]   # sums to T
HEAD1 = 16          # p3a coverage (groups 0-2)
HEAD2 = 32          # p3b covers [16,32) (group 3); b1h/b2h cover [0,32)
R = 8               # coefficient pattern repeat (24-elem packed runs)

F32 = mybir.dt.float32
BF16 = mybir.dt.bfloat16

_CACHE = {}


def _build_nc():
    nc = bacc.Bacc(None, target_bir_lowering=False)

    pts = nc.declare_dram_parameter("points", [B, NLOC, 3], F32, isOutput=False)
    nb = nc.declare_dram_parameter("neighborhoods", [B, NLOC, K, 3], F32, isOutput=False)
    out = nc.declare_dram_parameter("out", [B, NLOC, K, 3], BF16, isOutput=True)

    # partition = (b s), columns = t, free = 192 floats per point
    nbr = nb[:].rearrange("b (s t) k c -> (b s) t (k c)", s=S)
    outr = out[:].rearrange("b (s t) k c -> (b s) t (k c)", s=S)
    ptsr = pts[:].rearrange("b (s t) c -> (b s) (t c)", s=S)

    with TileContext(nc) as tc:
        with tc.tile_pool(name="const", bufs=1) as cpool, \
             tc.tile_pool(name="io_in", bufs=4) as inpool, \
             tc.tile_pool(name="io_out", bufs=4) as outpool, \
             tc.tile_pool(name="work", bufs=3) as wpool, \
             tc.tile_pool(name="npool", bufs=2) as npool:

            # ---- pre-data setup (runs in the boot window, free) ----
            dum = cpool.tile([P, 2], F32, tag="dum", name="dum")
            nc.vector.memset(dum[:][:, 0:1], 0.25)

            # ---- ACT: dummy sqrt first so the Sqrt table loads at boot ----
            nc.scalar.sqrt(out=dum[:][:, 1:2], in_=dum[:][:, 0:1])

            # n-pad slot 0 zero on ACT; touched on dum so the scheduler
            # cannot order it (and its Copy-table load) before the dummy
            # sqrt.  Slot 1 is zeroed later, behind the coefficient chain.
            zt0 = npool.tile([P, GMAX * K * 3], BF16, tag="n", name="nz0")
            nc.vector.tensor_copy(out=zt0[:][:, 0:1], in_=dum[:][:, 0:1])
            nc.scalar.memzero(zt0[:])
            zt1 = npool.tile([P, GMAX * K * 3], BF16, tag="n", name="nz1")

            # b1 = [cx, cx, 1]*R per column; z lanes set to 1.0 by small
            # strided DVE memsets, xy lanes written by ACT copies below.
            b1h = cpool.tile([P, HEAD2 * 3 * R], BF16, tag="b1h", name="b1h")
            b1t = cpool.tile([P, (T - HEAD2) * 3 * R], BF16, tag="b1t", name="b1t")
            b1hz = b1h[:].rearrange("p (t r c) -> p t r c", r=R, c=3)
            b1tz = b1t[:].rearrange("p (t r c) -> p t r c", r=R, c=3)
            nc.vector.memset(b1hz[:, :, :, 2], 1.0)
            nc.vector.memset(b1tz[:, :, :, 2], 1.0)

            # ---- points: fp32 via the first SWDGE issue ----
            pts_sb = cpool.tile([P, T * 3], F32, tag="pts", name="pts")
            nc.gpsimd.dma_start(out=pts_sb[:], in_=ptsr)
            pv = pts_sb[:].rearrange("p (t c) -> p t c", c=3)
            px = pv[:, :, 0]
            py = pv[:, :, 1]
            pz = pv[:, :, 2]

            def ctile(tag, w=1, dt=F32):
                return cpool.tile([P, T * w], dt, tag=tag, name=tag)

            sq3 = ctile("sq3", 3)
            r2 = ctile("r2")
            n2 = ctile("n2")
            nn = ctile("nn")
            ir2 = ctile("ir2")
            aa = ctile("aa")
            cx = ctile("cx")
            cy = ctile("cy")

            sqv = sq3[:].rearrange("p (t c) -> p t c", c=3)

            # ---- p3 = [px,py,pz]*R in three ACT pieces (a/b emitted
            # before the chain so ACT casts them while DVE squares) ----
            p3a = cpool.tile([P, HEAD1 * 3 * R], BF16, tag="p3a", name="p3a")
            p3b = cpool.tile([P, (HEAD2 - HEAD1) * 3 * R], BF16, tag="p3b", name="p3b")
            p3c = cpool.tile([P, (T - HEAD2) * 3 * R], BF16, tag="p3c", name="p3c")
            p3av = p3a[:].rearrange("p (t r c) -> p t r c", r=R, c=3)
            p3bv = p3b[:].rearrange("p (t r c) -> p t r c", r=R, c=3)
            p3cv = p3c[:].rearrange("p (t r c) -> p t r c", r=R, c=3)
            nc.scalar.copy(
                out=p3av[:],
                in_=pv[:, 0:HEAD1, None, :].broadcast_to([P, HEAD1, R, 3]),
            )
            nc.scalar.copy(
                out=p3bv[:],
                in_=pv[:, HEAD1:HEAD2, None, :].broadcast_to(
                    [P, HEAD2 - HEAD1, R, 3]),
            )

            p3aw = p3a[:].rearrange("p (t w) -> p t w", w=3 * R)
            p3bw = p3b[:].rearrange("p (t w) -> p t w", w=3 * R)
            p3cw = p3c[:].rearrange("p (t w) -> p t w", w=3 * R)

            def psrc_for(sl):
                if sl.stop <= HEAD1:
                    return p3aw[:, sl, :]
                if sl.stop <= HEAD2:
                    return p3bw[:, slice(sl.start - HEAD1, sl.stop - HEAD1), :]
                return p3cw[:, slice(sl.start - HEAD2, sl.stop - HEAD2), :]

            col_slices = []
            _t0 = 0
            for _G in GROUPS:
                col_slices.append(slice(_t0, _t0 + _G))
                _t0 += _G

            s_tiles = {}
            ot_tiles = {}

            def emit_load_sub(g, touch=None):
                sl = col_slices[g]
                G = sl.stop - sl.start
                nb_t = inpool.tile([P, G, K, 3], BF16, tag="nb", name=f"nb{g}")
                nc.gpsimd.dma_start(
                    out=nb_t[:].rearrange("p g k c -> p g (k c)"),
                    in_=nbr[:, sl, :],
                )
                s3 = wpool.tile([P, G, K, 3], BF16, tag="s", name=f"s{g}")
                if touch is not None:
                    # WAW pin: keeps this SUB from being statically ordered
                    # ahead of `touch`'s producer (the DMA cost model does
                    # not know SWDGE completions are slow).
                    nc.vector.tensor_copy(out=s3[:, 0:1, 0, 0], in_=touch)
                q12 = nb_t[:].rearrange("p g (kk r) c -> p g kk (r c)", r=R)
                s12 = s3[:].rearrange("p g (kk r) c -> p g kk (r c)", r=R)
                p3_b = psrc_for(sl)[:, :, None, :].broadcast_to(
                    [P, G, K // R, 3 * R])
                nc.vector.tensor_sub(out=s12[:], in0=q12[:], in1=p3_b)
                s_tiles[g] = s3

            # coefficient chain (fp32): r2 = px^2+py^2, n2 = r2+pz^2,
            # nn = sqrt(n2) on ACT, a = nn/r2, cx = px*a, cy = py*a.
            # Group-0/1 loads+SUBs are hoisted between ir2 and aa so the
            # in-order DVE stream has work while sqrt's cross-engine hop
            # (plus the zt1 memzero ahead of it on ACT) resolves.
            nc.vector.tensor_mul(out=sq3[:], in0=pv[:], in1=pv[:])
            nc.vector.tensor_add(out=r2[:], in0=sqv[:, :, 0], in1=sqv[:, :, 1])
            nc.vector.tensor_add(out=n2[:], in0=r2[:], in1=sqv[:, :, 2])
            nc.vector.reciprocal_approx_fast(out=ir2[:], in_=r2[:])
            nc.scalar.sqrt(out=nn[:], in_=n2[:])
            emit_load_sub(0)
            emit_load_sub(1)
            nc.vector.tensor_mul(out=aa[:], in0=nn[:], in1=ir2[:])
            nc.vector.tensor_mul(out=cx[:], in0=px, in1=aa[:])
            nc.vector.tensor_mul(out=cy[:], in0=py, in1=aa[:])

            # ---- b1/b2 head pieces on ACT (gate group 0-3 muls) ----
            b1hv = b1h[:].rearrange("p (t r c) -> p t r c", r=R, c=3)
            b1tv = b1t[:].rearrange("p (t r c) -> p t r c", r=R, c=3)
            nc.scalar.copy(
                out=b1hv[:, :, :, 0:2],
                in_=cx[:, 0:HEAD2, None, None].broadcast_to([P, HEAD2, R, 2]),
            )
            b2h = cpool.tile([P, HEAD2 * 2], BF16, tag="b2h", name="b2h")
            b2t = cpool.tile([P, (T - HEAD2) * 2], BF16, tag="b2t", name="b2t")
            b2hv = b2h[:].rearrange("p (t c) -> p t c", c=2)
            b2tv = b2t[:].rearrange("p (t c) -> p t c", c=2)
            nc.scalar.copy(out=b2hv[:, :, 0], in_=cy[:, 0:HEAD2])
            nc.scalar.mul(out=b2hv[:, :, 1], in_=cy[:, 0:HEAD2], mul=-1.0)

            # n-pad slot 1: zero once cx exists (ties with b1h resolve to
            # b1h by priority); needed by group 1's swap-mul ~16us.
            nc.vector.tensor_copy(out=zt1[:][:, 0:1], in_=cx[:][:, 0:1])
            nc.scalar.memzero(zt1[:])

            # p3c's only data dep is the points tile; a touch-write reading
            # cy makes it ready only after the coefficient chain, so the
            # static schedule keeps the cheap head pieces (b1h/b2h) ahead
            # of this 2us copy on ACT.
            nc.vector.tensor_copy(out=p3c[:][:, 0:1], in_=cy[:][:, 0:1])
            nc.scalar.copy(
                out=p3cv[:],
                in_=pv[:, HEAD2:T, None, :].broadcast_to([P, T - HEAD2, R, 3]),
            )

            # ---- b1/b2 tail pieces (needed from group 4 on) ----
            nc.scalar.copy(
                out=b1tv[:, :, :, 0:2],
                in_=cx[:, HEAD2:T, None, None].broadcast_to(
                    [P, T - HEAD2, R, 2]),
            )
            nc.scalar.copy(out=b2tv[:, :, 0], in_=cy[:, HEAD2:T])
            nc.scalar.mul(out=b2tv[:, :, 1], in_=cy[:, HEAD2:T], mul=-1.0)

            b1hw = b1h[:].rearrange("p (t w) -> p t w", w=3 * R)
            b1tw = b1t[:].rearrange("p (t w) -> p t w", w=3 * R)

            for g, G in enumerate(GROUPS):
                sl = col_slices[g]

                if g >= 2:
                    # fp32 -> bf16 cast in flight: SWDGE (gpsimd) DMA
                    emit_load_sub(
                        g, touch=ot_tiles[0][:, 0, 0, 0:1] if g == 2 else None)
                s3 = s_tiles[g]

                m3 = wpool.tile([P, G, K, 3], BF16, tag="m", name=f"m{g}")
                n3 = npool.tile([P, G, K, 3], BF16, tag="n", name=f"n{g}")
                ot = outpool.tile([P, G, K, 3], BF16, tag="ot", name=f"ot{g}")
                ot_tiles[g] = ot

                # regrouped views with contiguous 3R-elem inner runs
                s12 = s3[:].rearrange("p g (kk r) c -> p g kk (r c)", r=R)
                m12 = m3[:].rearrange("p g (kk r) c -> p g kk (r c)", r=R)
                n12 = n3[:].rearrange("p g (kk r) c -> p g kk (r c)", r=R)
                o12 = ot[:].rearrange("p g (kk r) c -> p g kk (r c)", r=R)

                # m3 = s3 * [cx, cx, 1]
                if sl.stop <= HEAD2:
                    b1src = b1hw[:, sl, :]
                    b2src = b2hv[:, sl, :]
                else:
                    tsl = slice(sl.start - HEAD2, sl.stop - HEAD2)
                    b1src = b1tw[:, tsl, :]
                    b2src = b2tv[:, tsl, :]
                b1_b = b1src[:, :, None, :].broadcast_to(
                    [P, G, K // R, 3 * R])
                nc.vector.tensor_mul(out=m12[:], in0=s12[:], in1=b1_b)

                # n3_xy = [sy, sx] * [cy, -cy]; z lanes stay zero
                b2_b = b2src[:, :, None, :].broadcast_to([P, G, K, 2])
                nc.vector.tensor_mul(
                    out=n3[:, :, :, 0:2], in0=s3[:, :, :, 1::-1], in1=b2_b,
                )

                # ot = m3 + n3  (full stream; out_z = s_z + 0)
                nc.vector.tensor_add(out=o12[:], in0=m12[:], in1=n12[:])

                # out-DMA on the ACT HWDGE ring (input stream is SWDGE)
                nc.scalar.dma_start(
                    out=outr[:, sl, :],
                    in_=ot[:].rearrange("p g k c -> p g (k c)"),
                )

    nc.compile()
    return nc


def _get_nc():
    if "nc" not in _CACHE:
        _CACHE["nc"] = _build_nc()
    return _CACHE["nc"]


def kernel(points, neighborhoods):
    pts = np.ascontiguousarray(np.asarray(points, dtype=np.float32))
    nb = np.ascontiguousarray(np.asarray(neighborhoods, dtype=np.float32))
    assert pts.shape == (B, N, 3), pts.shape
    assert nb.shape == (B, N, K, 3), nb.shape

    in_maps = []
    for c in range(NCORES):
        sl = slice(c * NLOC, (c + 1) * NLOC)
        in_maps.append({
            "points": np.ascontiguousarray(pts[:, sl]),
            "neighborhoods": np.ascontiguousarray(nb[:, sl]),
        })

    res = run_bass_kernel_spmd(_get_nc(), in_maps, list(range(NCORES))).results
    out = np.concatenate(
        [np.asarray(res[c]["out"]).astype(np.float32) for c in range(NCORES)],
        axis=1,
    )
    return out


# revision 14
# speedup vs baseline: 1.0278x; 1.0278x over previous
"""Trainium2 Bass kernel for NeighborhoodNormalization.

Math: the reference builds a per-point homogeneous transform
T = [[ux,-uy,0,px],[uy,ux,0,py],[0,0,1,pz],[0,0,0,1]] (u = p/||p||),
inverts it, and applies it to 64 neighbors per point.  Closed form with
r2 = px^2+py^2, n = ||p||, a = n/r2, cx = px*a, cy = py*a, s = q - p:

    out.x =  cx*sx + cy*sy
    out.y = -cy*sx + cx*sy
    out.z =  sz

Pure data parallel over the N=8192 point axis across 8 cores.

Per-core layout: 16384 points = 128 partitions x 128 columns, partition
p = b*8 + s holds points with local n = s*128 + t.  Neighbor rows stay
contiguous in HBM per point (64*3 floats), so DMAs move [128 x W*768B]
blocks.

Steady state is DVE-bound: 4 tensor_tensor passes per group (sub, mul,
swap-mul, add) in bf16; sub/mul/add hit the 2x packed rate, the xy-swap
pass runs at ~0.8 cyc/elem (2-elem reversed runs can't pack).  Input
arrives via SWDGE fp32->bf16 cast DMA (~358 GB/s HBM cap); steady input
demand is ~320 GB/s so DMA trails just under the ceiling.

Head optimization (vs the 18us steady-start of the earlier version):
 - points load fp32 on the sync HWDGE ring, in parallel with the first
   SWDGE neighbor loads (also: fp32 coefficient math, better accuracy);
 - a dummy sqrt is the first ACT op so the Sqrt activation table loads
   during boot instead of on the critical path;
 - the coefficient chain is compressed (one full-vector square) and all
   b1/b2 coefficient-tile builds run on ACT, split into head (cols
   0-32) / tail pieces so group 0's mul only waits for the head piece;
 - b1's z=1 lanes and the n-pad zero slots are DVE memsets in the
   pre-data boot window (DVE is idle until points land);
 - groups [4,4,8,16*6,12,2,2]: small leading groups start compute as
   soon as ~0.4MB of input has landed; tiny trailing groups shorten the
   final output-DMA drain.
"""

import sys

if "/opt/trn_rl_repo" not in sys.path:
    sys.path.insert(0, "/opt/trn_rl_repo")

import numpy as np

import concourse.bass as bass
import concourse.bacc as bacc
import concourse.mybir as mybir
from concourse.tile import TileContext
from concourse.bass_utils import run_bass_kernel_spmd

B = 16
N = 8192
K = 64
NCORES = 8
NLOC = N // NCORES  # 1024 points per core
P = 128             # SBUF partitions
S = NLOC // P       # 8 partition sub-blocks per batch entry
T = (B * NLOC) // P  # 128 point-columns per partition
GMAX = 16
GROUPS = [4, 4, 8] + [16] * 6 + [12, 2, 2]   # sums to T
HEAD1 = 16          # p3a coverage (groups 0-2)
HEAD2 = 32          # p3b covers [16,32) (group 3); b1h/b2h cover [0,32)
R = 8               # coefficient pattern repeat (24-elem packed runs)

F32 = mybir.dt.float32
BF16 = mybir.dt.bfloat16

_CACHE = {}


def _build_nc():
    nc = bacc.Bacc(None, target_bir_lowering=False)

    pts = nc.declare_dram_parameter("points", [B, NLOC, 3], F32, isOutput=False)
    nb = nc.declare_dram_parameter("neighborhoods", [B, NLOC, K, 3], F32, isOutput=False)
    out = nc.declare_dram_parameter("out", [B, NLOC, K, 3], BF16, isOutput=True)

    # partition = (b s), columns = t, free = 192 floats per point
    nbr = nb[:].rearrange("b (s t) k c -> (b s) t (k c)", s=S)
    outr = out[:].rearrange("b (s t) k c -> (b s) t (k c)", s=S)
    ptsr = pts[:].rearrange("b (s t) c -> (b s) (t c)", s=S)

    with TileContext(nc) as tc:
        with tc.tile_pool(name="const", bufs=1) as cpool, \
             tc.tile_pool(name="io_in", bufs=4) as inpool, \
             tc.tile_pool(name="io_out", bufs=4) as outpool, \
             tc.tile_pool(name="work", bufs=3) as wpool, \
             tc.tile_pool(name="npool", bufs=2) as npool:

            # ---- pre-data setup (runs in the boot window, free) ----
            dum = cpool.tile([P, 2], F32, tag="dum", name="dum")
            nc.vector.memset(dum[:][:, 0:1], 0.25)

            # ---- ACT: dummy sqrt first so the Sqrt table loads at boot ----
            nc.scalar.sqrt(out=dum[:][:, 1:2], in_=dum[:][:, 0:1])

            # n-pad slot 0 zero on ACT; touched on dum so the scheduler
            # cannot order it (and its Copy-table load) before the dummy
            # sqrt.  Slot 1 is zeroed later, behind the coefficient chain.
            zt0 = npool.tile([P, GMAX * K * 3], BF16, tag="n", name="nz0")
            nc.vector.tensor_copy(out=zt0[:][:, 0:1], in_=dum[:][:, 0:1])
            nc.scalar.memzero(zt0[:])
            zt1 = npool.tile([P, GMAX * K * 3], BF16, tag="n", name="nz1")

            # b1 = [cx, cx, 1]*R per column; z lanes set to 1.0 by small
            # strided DVE memsets, xy lanes written by ACT copies below.
            b1h = cpool.tile([P, HEAD2 * 3 * R], BF16, tag="b1h", name="b1h")
            b1t = cpool.tile([P, (T - HEAD2) * 3 * R], BF16, tag="b1t", name="b1t")
            b1hz = b1h[:].rearrange("p (t r c) -> p t r c", r=R, c=3)
            b1tz = b1t[:].rearrange("p (t r c) -> p t r c", r=R, c=3)
            nc.vector.memset(b1hz[:, :, :, 2], 1.0)
            nc.vector.memset(b1tz[:, :, :, 2], 1.0)

            # ---- points: fp32 via the first SWDGE issue ----
            pts_sb = cpool.tile([P, T * 3], F32, tag="pts", name="pts")
            nc.gpsimd.dma_start(out=pts_sb[:], in_=ptsr)
            pv = pts_sb[:].rearrange("p (t c) -> p t c", c=3)
            px = pv[:, :, 0]
            py = pv[:, :, 1]
            pz = pv[:, :, 2]

            def ctile(tag, w=1, dt=F32):
                return cpool.tile([P, T * w], dt, tag=tag, name=tag)

            sq3 = ctile("sq3", 3)
            r2 = ctile("r2")
            n2 = ctile("n2")
            nn = ctile("nn")
            ir2 = ctile("ir2")
            aa = ctile("aa")
            cx = ctile("cx")
            cy = ctile("cy")

            sqv = sq3[:].rearrange("p (t c) -> p t c", c=3)

            # ---- p3 = [px,py,pz]*R in three ACT pieces (a/b emitted
            # before the chain so ACT casts them while DVE squares) ----
            p3a = cpool.tile([P, HEAD1 * 3 * R], BF16, tag="p3a", name="p3a")
            p3b = cpool.tile([P, (HEAD2 - HEAD1) * 3 * R], BF16, tag="p3b", name="p3b")
            p3c = cpool.tile([P, (T - HEAD2) * 3 * R], BF16, tag="p3c", name="p3c")
            p3av = p3a[:].rearrange("p (t r c) -> p t r c", r=R, c=3)
            p3bv = p3b[:].rearrange("p (t r c) -> p t r c", r=R, c=3)
            p3cv = p3c[:].rearrange("p (t r c) -> p t r c", r=R, c=3)
            nc.scalar.copy(
                out=p3av[:],
                in_=pv[:, 0:HEAD1, None, :].broadcast_to([P, HEAD1, R, 3]),
            )
            nc.scalar.copy(
                out=p3bv[:],
                in_=pv[:, HEAD1:HEAD2, None, :].broadcast_to(
                    [P, HEAD2 - HEAD1, R, 3]),
            )

            p3aw = p3a[:].rearrange("p (t w) -> p t w", w=3 * R)
            p3bw = p3b[:].rearrange("p (t w) -> p t w", w=3 * R)
            p3cw = p3c[:].rearrange("p (t w) -> p t w", w=3 * R)

            def psrc_for(sl):
                if sl.stop <= HEAD1:
                    return p3aw[:, sl, :]
                if sl.stop <= HEAD2:
                    return p3bw[:, slice(sl.start - HEAD1, sl.stop - HEAD1), :]
                return p3cw[:, slice(sl.start - HEAD2, sl.stop - HEAD2), :]

            col_slices = []
            _t0 = 0
            for _G in GROUPS:
                col_slices.append(slice(_t0, _t0 + _G))
                _t0 += _G

            s_tiles = {}

            def emit_load_sub(g, touch=None):
                sl = col_slices[g]
                G = sl.stop - sl.start
                nb_t = inpool.tile([P, G, K, 3], BF16, tag="nb", name=f"nb{g}")
                nc.gpsimd.dma_start(
                    out=nb_t[:].rearrange("p g k c -> p g (k c)"),
                    in_=nbr[:, sl, :],
                )
                s3 = wpool.tile([P, G, K, 3], BF16, tag="s", name=f"s{g}")
                if touch is not None:
                    # WAW pin: keeps this SUB from being statically ordered
                    # ahead of `touch`'s producer (the DMA cost model does
                    # not know SWDGE completions are slow).
                    nc.vector.tensor_copy(out=s3[:, 0:1, 0, 0], in_=touch)
                q12 = nb_t[:].rearrange("p g (kk r) c -> p g kk (r c)", r=R)
                s12 = s3[:].rearrange("p g (kk r) c -> p g kk (r c)", r=R)
                p3_b = psrc_for(sl)[:, :, None, :].broadcast_to(
                    [P, G, K // R, 3 * R])
                nc.vector.tensor_sub(out=s12[:], in0=q12[:], in1=p3_b)
                s_tiles[g] = s3

            # coefficient chain (fp32): r2 = px^2+py^2, n2 = r2+pz^2,
            # nn = sqrt(n2) on ACT, a = nn/r2, cx = px*a, cy = py*a.
            # Group-0/1 loads+SUBs are hoisted between ir2 and aa so the
            # in-order DVE stream has work while sqrt's cross-engine hop
            # (plus the zt1 memzero ahead of it on ACT) resolves.
            nc.vector.tensor_mul(out=sq3[:], in0=pv[:], in1=pv[:])
            nc.vector.tensor_add(out=r2[:], in0=sqv[:, :, 0], in1=sqv[:, :, 1])
            nc.vector.tensor_add(out=n2[:], in0=r2[:], in1=sqv[:, :, 2])
            nc.vector.reciprocal_approx_fast(out=ir2[:], in_=r2[:])
            nc.scalar.sqrt(out=nn[:], in_=n2[:])
            emit_load_sub(0)
            emit_load_sub(1)
            nc.vector.tensor_mul(out=aa[:], in0=nn[:], in1=ir2[:])
            nc.vector.tensor_mul(out=cx[:], in0=px, in1=aa[:])
            nc.vector.tensor_mul(out=cy[:], in0=py, in1=aa[:])

            # ---- b1/b2 head pieces on ACT (gate group 0-3 muls) ----
            b1hv = b1h[:].rearrange("p (t r c) -> p t r c", r=R, c=3)
            b1tv = b1t[:].rearrange("p (t r c) -> p t r c", r=R, c=3)
            nc.scalar.copy(
                out=b1hv[:, :, :, 0:2],
                in_=cx[:, 0:HEAD2, None, None].broadcast_to([P, HEAD2, R, 2]),
            )
            b2h = cpool.tile([P, HEAD2 * 2], BF16, tag="b2h", name="b2h")
            b2t = cpool.tile([P, (T - HEAD2) * 2], BF16, tag="b2t", name="b2t")
            b2hv = b2h[:].rearrange("p (t c) -> p t c", c=2)
            b2tv = b2t[:].rearrange("p (t c) -> p t c", c=2)
            nc.scalar.copy(out=b2hv[:, :, 0], in_=cy[:, 0:HEAD2])
            nc.scalar.mul(out=b2hv[:, :, 1], in_=cy[:, 0:HEAD2], mul=-1.0)

            # n-pad slot 1: zero once cx exists (ties with b1h resolve to
            # b1h by priority); needed by group 1's swap-mul ~16us.
            nc.vector.tensor_copy(out=zt1[:][:, 0:1], in_=cx[:][:, 0:1])
            nc.scalar.memzero(zt1[:])

            # p3c's only data dep is the points tile; a touch-write reading
            # cy makes it ready only after the coefficient chain, so the
            # static schedule keeps the cheap head pieces (b1h/b2h) ahead
            # of this 2us copy on ACT.
            nc.vector.tensor_copy(out=p3c[:][:, 0:1], in_=cy[:][:, 0:1])
            nc.scalar.copy(
                out=p3cv[:],
                in_=pv[:, HEAD2:T, None, :].broadcast_to([P, T - HEAD2, R, 3]),
            )

            # ---- b1/b2 tail pieces (needed from group 4 on) ----
            nc.scalar.copy(
                out=b1tv[:, :, :, 0:2],
                in_=cx[:, HEAD2:T, None, None].broadcast_to(
                    [P, T - HEAD2, R, 2]),
            )
            nc.scalar.copy(out=b2tv[:, :, 0], in_=cy[:, HEAD2:T])
            nc.scalar.mul(out=b2tv[:, :, 1], in_=cy[:, HEAD2:T], mul=-1.0)

            b1hw = b1h[:].rearrange("p (t w) -> p t w", w=3 * R)
            b1tw = b1t[:].rearrange("p (t w) -> p t w", w=3 * R)

            for g, G in enumerate(GROUPS):
                sl = col_slices[g]

                if g >= 2:
                    # fp32 -> bf16 cast in flight: SWDGE (gpsimd) DMA
                    emit_load_sub(g, touch=cy[:, 0:1] if g == 2 else None)
                s3 = s_tiles[g]

                m3 = wpool.tile([P, G, K, 3], BF16, tag="m", name=f"m{g}")
                n3 = npool.tile([P, G, K, 3], BF16, tag="n", name=f"n{g}")
                ot = outpool.tile([P, G, K, 3], BF16, tag="ot", name=f"ot{g}")

                # regrouped views with contiguous 3R-elem inner runs
                s12 = s3[:].rearrange("p g (kk r) c -> p g kk (r c)", r=R)
                m12 = m3[:].rearrange("p g (kk r) c -> p g kk (r c)", r=R)
                n12 = n3[:].rearrange("p g (kk r) c -> p g kk (r c)", r=R)
                o12 = ot[:].rearrange("p g (kk r) c -> p g kk (r c)", r=R)

                # m3 = s3 * [cx, cx, 1]
                if sl.stop <= HEAD2:
                    b1src = b1hw[:, sl, :]
                    b2src = b2hv[:, sl, :]
                else:
                    tsl = slice(sl.start - HEAD2, sl.stop - HEAD2)
                    b1src = b1tw[:, tsl, :]
                    b2src = b2tv[:, tsl, :]
                b1_b = b1src[:, :, None, :].broadcast_to(
                    [P, G, K // R, 3 * R])
                nc.vector.tensor_mul(out=m12[:], in0=s12[:], in1=b1_b)

                # n3_xy = [sy, sx] * [cy, -cy]; z lanes stay zero
                b2_b = b2src[:, :, None, :].broadcast_to([P, G, K, 2])
                nc.vector.tensor_mul(
                    out=n3[:, :, :, 0:2], in0=s3[:, :, :, 1::-1], in1=b2_b,
                )

                # ot = m3 + n3  (full stream; out_z = s_z + 0)
                nc.vector.tensor_add(out=o12[:], in0=m12[:], in1=n12[:])

                # out-DMA on the ACT HWDGE ring (input stream is SWDGE)
                nc.scalar.dma_start(
                    out=outr[:, sl, :],
                    in_=ot[:].rearrange("p g k c -> p g (k c)"),
                )

    nc.compile()
    return nc


def _get_nc():
    if "nc" not in _CACHE:
        _CACHE["nc"] = _build_nc()
    return _CACHE["nc"]


def kernel(points, neighborhoods):
    pts = np.ascontiguousarray(np.asarray(points, dtype=np.float32))
    nb = np.ascontiguousarray(np.asarray(neighborhoods, dtype=np.float32))
    assert pts.shape == (B, N, 3), pts.shape
    assert nb.shape == (B, N, K, 3), nb.shape

    in_maps = []
    for c in range(NCORES):
        sl = slice(c * NLOC, (c + 1) * NLOC)
        in_maps.append({
            "points": np.ascontiguousarray(pts[:, sl]),
            "neighborhoods": np.ascontiguousarray(nb[:, sl]),
        })

    res = run_bass_kernel_spmd(_get_nc(), in_maps, list(range(NCORES))).results
    out = np.concatenate(
        [np.asarray(res[c]["out"]).astype(np.float32) for c in range(NCORES)],
        axis=1,
    )
    return out
